# revision 36
# baseline (speedup 1.0000x reference)
"""Trainium2 Bass kernel for a sliding-window self-attention block.

The reference network applies softmax over a singleton axis, so the attention
weights are identically 1.0 and the whole module reduces to:

    h   = relu((x + pos_enc) @ W1 + b1)            # [B, S, 64]
    p   = h @ (Wv @ W2)                            # [B, S, 2]
    out = sliding_window_sum(p, +-8, zero-pad) + b2
    weights = ones([B, S, 1, 17])

Sharding: data-parallel over batch, 2 batches per core on 8 NeuronCores.

Per-core dataflow (batches b0/b1 processed jointly on 128 partitions):
  - 4 big DMAs load x and pos_enc (1 MB each, both batches folded).
  - DVE adds -> h0 in [seq_on_partitions, (j, b, feat)] layout (float32r).
  - PE transposes [128 seq, (b,f)=128] tiles -> PSUM [(b,f), seq] (bf16
    identity as the moving operand: 1 cycle/row).
  - One block-diagonal f32r matmul per 512-chunk applies W1 to both batches,
    ACT relu(+b1) -> one more block-diag matmul applies Wc = Wv@W2 -> p^T.
  - p^T chunks gather into a zero-padded [4, 4240] buffer; 3 SBUF DMAs
    re-partition it into a [128(b,f,c), 144] halo layout; 5 log-tree DVE
    adds compute the 17-wide window sum; ACT adds b2; one DMA stores
    [4, 4096] = out^T per batch (host transposes the tiny result).
"""

import numpy as np

B, S, I, H, O = 16, 4096, 64, 64, 2
A = 8                 # atten_size; window = 2*A+1 = 17
NCORES = 8
BPC = B // NCORES     # batches per core = 2
CHUNK = 512           # seq positions per PSUM-stage chunk
NCHUNK = S // CHUNK   # 8
GCHUNK = 2048         # seq positions per DMA load chunk
NG = S // GCHUNK      # 2
JG = GCHUNK // 128    # 16 transpose tiles per load chunk
PAD = 4240            # 8 (left zero pad) + 4096 + 136 (right pad/tail slack)

_PROGRAM = None


def _build_program():
    import concourse.bacc as bacc
    import concourse.mybir as mybir
    from concourse.bass_types import AP as bass_AP
    from concourse.tile import TileContext

    f32 = mybir.dt.float32
    f32r = mybir.dt.float32r
    bf16 = mybir.dt.bfloat16

    nc = bacc.Bacc()

    x_d = nc.declare_dram_parameter("x", [BPC, S, I], f32, isOutput=False)
    p_d = nc.declare_dram_parameter("p", [BPC, S, I], f32, isOutput=False)
    c_d = nc.declare_dram_parameter("consts", [128, 262], f32r, isOutput=False)
    o_d = nc.declare_dram_parameter("o", [2 * BPC, S], f32, isOutput=True)
    pdr = nc.dram_tensor("p_bounce", [2 * BPC, PAD], f32)

    with TileContext(nc) as tc:
        with (
            tc.tile_pool(name="const", bufs=1) as const,
            tc.tile_pool(name="inp", bufs=2) as inp,
            tc.tile_pool(name="hsb", bufs=4) as hsb,
            tc.tile_pool(name="pp", bufs=1) as pp,
            tc.tile_pool(name="wsum", bufs=2) as wsum,
            tc.tile_pool(name="ps_t", bufs=2, space="PSUM") as ps_t,
            tc.tile_pool(name="ps_h", bufs=3, space="PSUM") as ps_h,
            tc.tile_pool(name="ps_p", bufs=2, space="PSUM") as ps_p,
            tc.tile_pool(name="ps_w", bufs=1, space="PSUM") as ps_w,
        ):
            c_t = const.tile([128, 262], f32r)
            nc.sync.dma_start(out=c_t[:], in_=c_d[:])
            w1_t = c_t[:, 0:128]
            wc_t = c_t[:, 128 : 128 + 2 * BPC]
            id_t = c_t[:, 132:260]                    # [128, 128] f32r identity
            b1_t = c_t[:, 260:261].bitcast(f32)
            b2_t = c_t[:, 261:262].bitcast(f32)

            # p_pad[(2b+f), 8 + s] = p^T values; zero pads at both ends so the
            # halo gather below never needs edge cases.
            # Warm up the tensor engine's HAM clock gate while the first
            # loads are in flight: dummy matmuls on the consts tile keep PE
            # busy past the 3.4us activity window so real matmuls run at
            # 2.4 GHz instead of 1.2.
            wps = ps_w.tile([128, 256], f32)
            for _ in range(16):
                nc.tensor.matmul(
                    out=wps[:], lhsT=c_t[:, 0:128], rhs=c_t[:, 0:256],
                    start=True, stop=True,
                )

            p_pad = pp.tile([2 * BPC, PAD], f32)
            nc.vector.memset(p_pad[:, 0:8], 0.0)
            nc.vector.memset(p_pad[:, 8 + S : PAD], 0.0)
            q_t = pp.tile([128, 144], f32)
            nc.vector.memset(q_t[:, :], 0.0)

            for g in range(NG):
                s0 = g * GCHUNK
                # Raw layout: partition q holds 16 consecutive seq rows
                # (4 KB contiguous per partition -> cheap DMA descriptors).
                # Element (q, b, 64u + f) = x[b, s0 + 16q + u, f].
                xt = inp.tile([128, BPC, GCHUNK // 2], f32, tag="xt")
                pt = inp.tile([128, BPC, GCHUNK // 2], f32, tag="pt")
                # Half-granularity loads (256 KB each); the h=1 half is
                # emitted after the first chunks so the h=0 transfers aren't
                # competing for DMA engines with data needed later.
                HC = GCHUNK // 4  # 1024 columns = 512 seq rows worth per half
                def load_half(h):
                    for b in range(BPC):
                        nc.sync.dma_start(
                            out=xt[:, b, h * HC : (h + 1) * HC],
                            in_=x_d[b, s0 : s0 + GCHUNK, :].rearrange(
                                "(q v) f -> q (v f)", q=128
                            )[:, h * HC : (h + 1) * HC],
                        )
                        nc.sync.dma_start(
                            out=pt[:, b, h * HC : (h + 1) * HC],
                            in_=p_d[b, s0 : s0 + GCHUNK, :].rearrange(
                                "(q v) f -> q (v f)", q=128
                            )[:, h * HC : (h + 1) * HC],
                        )
                load_half(0)
                # h0c free layout (u, b, f): the (b, f) pair of each u-slice is
                # contiguous, so the transpose stationary is a single free dim.
                # Adds are split per 4-u-slice group so downstream transposes
                # unblock early; batch 0 on DVE, batch 1 on GpSimd.
                h0c = inp.tile([128, JG, BPC, I], f32r, tag="h0")
                def add_kk(kk):
                    js = slice(4 * kk, 4 * kk + 4)
                    nc.vector.tensor_add(
                        out=h0c[:, js, 0, :],
                        in0=xt[:, 0].rearrange("q (v f) -> q v f", f=I)[:, js],
                        in1=pt[:, 0].rearrange("q (v f) -> q v f", f=I)[:, js],
                    )
                    nc.gpsimd.tensor_add(
                        out=h0c[:, js, 1, :],
                        in0=xt[:, 1].rearrange("q (v f) -> q v f", f=I)[:, js],
                        in1=pt[:, 1].rearrange("q (v f) -> q v f", f=I)[:, js],
                    )
                add_kk(0)
                add_kk(1)

                for kk in range(GCHUNK // CHUNK):
                    if kk == 1:
                        load_half(1)
                    if kk == 2:
                        add_kk(2)
                        add_kk(3)
                    # u-slices 4kk..4kk+3; transpose input [128, (b, f)] whose
                    # column q maps to seq s0 + 16q + u.
                    h0T_ps = ps_t.tile([128, CHUNK], f32r)
                    for ul in range(CHUNK // 128):
                        u = 4 * kk + ul
                        nc.tensor.transpose(
                            out=h0T_ps[:, 128 * ul : 128 * ul + 128],
                            in_=h0c[:, u].rearrange("p b f -> p (b f)"),
                            identity=id_t[:],
                        )
                    h0T = hsb.tile([128, CHUNK], f32r, tag="h0T")
                    nc.scalar.copy(out=h0T[:], in_=h0T_ps[:])

                    hT_ps = ps_h.tile([128, CHUNK], f32)
                    nc.tensor.matmul(
                        out=hT_ps[:], lhsT=w1_t, rhs=h0T[:], start=True, stop=True
                    )
                    hT = hsb.tile([128, CHUNK], f32r, tag="hT")
                    nc.scalar.activation(
                        out=hT[:],
                        in_=hT_ps[:],
                        func=mybir.ActivationFunctionType.Relu,
                        bias=b1_t,
                    )

                    pT_ps = ps_p.tile([2 * BPC, CHUNK], f32)
                    nc.tensor.matmul(
                        out=pT_ps[:], lhsT=wc_t, rhs=hT[:], start=True, stop=True
                    )
                    # Un-permute while scattering: pT col (ul, q) is seq
                    # s0 + 16q + 4kk + ul -> p_pad col base + 16q + ul.
                    base = 8 + s0 + 4 * kk
                    dst = p_pad[:, base : base + 2048].rearrange(
                        "p (pp u) -> p u pp", u=16
                    )[:, 0:4, :]
                    src = pT_ps.rearrange("p (u q) -> p u q", q=128)
                    nc.vector.tensor_copy(out=dst, in_=src)

                # Bounce this half of p_pad to DRAM so the halo gather below
                # can use one overlapping-window AP in (c, bf, u) order.
                # Split at col 2048 = an SBUF-bank boundary, so the g=0 dump
                # does not falsely depend on g=1's p_pad writes.
                lo = 2048 * g
                hi = 2048 + 2192 * g
                nc.scalar.dma_start(out=pdr[:, lo:hi], in_=p_pad[:, lo:hi])

                # Window sum for the c-chunks fully covered by data so far:
                # part 0 (c in [0,15)) runs overlapped with the g=1 compute.
                # Engine ops need 32-aligned partition starts, so part 0
                # computes rows [0:64) (rows 60-63 garbage, not stored) and
                # part 1 recomputes rows [32:60) along with its own.
                c0, c1 = (0, 15) if g == 0 else (15, 32)
                r = slice(4 * c0, 4 * c1)          # gathered + stored rows
                ro = slice(0, 64) if g == 0 else slice(0, 128)  # compute rows
                # Q[(c,bf), u] = p[bf, 128c + u - 8] (zero-padded via pdr pads)
                nc.scalar.dma_start(
                    out=q_t[r, :],
                    in_=bass_AP(
                        tensor=pdr.ap().tensor,
                        offset=128 * c0,
                        ap=[[128, c1 - c0], [PAD, 2 * BPC], [1, 144]],
                    ),
                )
                # 17-wide window sum: ws[u] = sum_{d=0..16} Q[u+d].
                t2 = wsum.tile([128, 143], f32, tag="t2")
                nc.vector.tensor_add(
                    out=t2[ro, :], in0=q_t[ro, 0:143], in1=q_t[ro, 1:144]
                )
                t4 = wsum.tile([128, 141], f32, tag="t4")
                nc.vector.tensor_add(
                    out=t4[ro, :], in0=t2[ro, 0:141], in1=t2[ro, 2:143]
                )
                t8 = wsum.tile([128, 137], f32, tag="t8")
                nc.vector.tensor_add(
                    out=t8[ro, :], in0=t4[ro, 0:137], in1=t4[ro, 4:141]
                )
                t16 = wsum.tile([128, 129], f32, tag="t16")
                nc.vector.tensor_add(
                    out=t16[ro, :], in0=t8[ro, 0:129], in1=t8[ro, 8:137]
                )
                ws_t = wsum.tile([128, 128], f32, tag="ws")
                nc.vector.tensor_add(
                    out=ws_t[ro, :], in0=t16[ro, 0:128], in1=q_t[ro, 16:144]
                )
                ows = wsum.tile([128, 128], f32, tag="ows")
                nc.scalar.activation(
                    out=ows[ro, :],
                    in_=ws_t[ro, :],
                    func=mybir.ActivationFunctionType.Identity,
                    bias=b2_t[ro],
                )
                nc.scalar.dma_start(
                    out=o_d[:, 128 * c0 : 128 * c1].rearrange(
                        "bf (c u) -> c bf u", u=128
                    ),
                    in_=ows[r, :],
                )

    nc.finalize()
    return nc


def _get_program():
    global _PROGRAM
    if _PROGRAM is None:
        _PROGRAM = _build_program()
    return _PROGRAM


def _host_inputs(W1, b1, Wv, W2, b2):
    """Pack the small replicated parameters into one [128, 198] f32 tensor."""
    W1 = np.asarray(W1, np.float32).reshape(I, H)
    Wc = (
        np.asarray(Wv, np.float32).reshape(H, H)
        @ np.asarray(W2, np.float32).reshape(H, O)
    ).astype(np.float32)
    consts = np.zeros((128, 262), np.float32)
    consts[:64, 0:64] = W1
    consts[64:, 64:128] = W1
    consts[:64, 128 : 128 + O] = Wc
    consts[64:, 128 + O : 128 + 2 * O] = Wc
    consts[:, 132:260] = np.eye(128, dtype=np.float32)
    consts[:, 260] = np.tile(np.asarray(b1, np.float32).reshape(H), BPC)
    # winsum partition layout is (c, bf) with bf = 2b + f -> f = p % 2
    pidx = np.arange(128)
    consts[:, 261] = np.asarray(b2, np.float32).reshape(O)[pidx % 2]
    return consts


def _in_maps(x, pos_enc, W1, b1, Wv, W2, b2):
    x = np.asarray(x, np.float32)
    pos_enc = np.asarray(pos_enc, np.float32)
    consts = _host_inputs(W1, b1, Wv, W2, b2)
    in_maps = []
    for r in range(NCORES):
        sl = slice(r * BPC, (r + 1) * BPC)
        in_maps.append(
            {
                "x": np.ascontiguousarray(x[sl]),
                "p": np.ascontiguousarray(pos_enc[sl]),
                "consts": consts,
            }
        )
    return in_maps


def _assemble(results):
    out = np.empty((B, S, O), np.float32)
    for r in range(NCORES):
        o = np.asarray(results[r]["o"]).reshape(BPC, O, S)
        out[r * BPC : (r + 1) * BPC] = o.transpose(0, 2, 1)
    weights = np.ones((B, S, 1, 2 * A + 1), np.float32)
    return out, weights


def kernel(x, pos_enc, W1, b1, Wq, Wk, Wv, W2, b2):
    from concourse.bass_utils import run_bass_kernel_spmd

    in_maps = _in_maps(x, pos_enc, W1, b1, Wv, W2, b2)
    nc = _get_program()
    res = run_bass_kernel_spmd(nc, in_maps, list(range(NCORES))).results
    return _assemble(res)


# revision 40
# speedup vs baseline: 1.1351x; 1.1351x over previous
"""Trainium2 Bass kernel for a sliding-window self-attention block.

The reference network applies softmax over a singleton axis, so the attention
weights are identically 1.0 and the whole module reduces to:

    h   = relu((x + pos_enc) @ W1 + b1)            # [B, S, 64]
    p   = h @ (Wv @ W2)                            # [B, S, 2]
    out = sliding_window_sum(p, +-8, zero-pad) + b2
    weights = ones([B, S, 1, 17])

Sharding: data-parallel over batch, 2 batches per core on 8 NeuronCores.

Per-core dataflow (batches b0/b1 processed jointly on 128 partitions):
  - Raw-layout loads (4 KB contiguous per partition -> cheap DMA
    descriptors); element (q, b, 64u+f) = x[b, 16q + u, f].
  - DVE/GpSimd adds -> h0 in f32r, free layout (u, b, f).
  - PE warmup matmuls run during the load phase to release the HAM clock
    gate before real work arrives.
  - PE transposes [128, (b,f)] u-slices -> PSUM [(b,f), cols], col q of
    u-slice u being seq 16q + u; DVE casts PSUM -> SBUF.
  - One block-diagonal f32r matmul per 512-col chunk applies W1 to both
    batches; ACT relu(+b1); second block-diag matmul applies Wc = Wv@W2.
  - p^T chunks scatter (un-permuting seq) into a zero-padded [4, 4240]
    buffer; 3 SBUF DMAs re-partition it into a [(b,f,c), 144] halo
    layout; 5 log-tree DVE adds compute the 17-wide window sum; ACT adds
    b2; one DMA stores [4, 4096] = out^T per batch (host transposes).
"""

import numpy as np

B, S, I, H, O = 16, 4096, 64, 64, 2
A = 8                 # atten_size; window = 2*A+1 = 17
NCORES = 8
BPC = B // NCORES     # batches per core = 2
CHUNK = 512           # machine columns per PSUM-stage chunk
GCHUNK = 2048         # seq positions per load group
NG = S // GCHUNK      # 2
JG = GCHUNK // 128    # 16 u-slices per load group
PAD = 4240            # 8 zero pad + 4096 + 136 tail pad

_PROGRAM = None


def _build_program():
    import concourse.bacc as bacc
    import concourse.mybir as mybir
    from concourse.tile import TileContext

    f32 = mybir.dt.float32
    f32r = mybir.dt.float32r

    nc = bacc.Bacc()

    x_d = nc.declare_dram_parameter("x", [BPC, S, I], f32, isOutput=False)
    p_d = nc.declare_dram_parameter("p", [BPC, S, I], f32, isOutput=False)
    c_d = nc.declare_dram_parameter("consts", [128, 262], f32r, isOutput=False)
    o_d = nc.declare_dram_parameter("o", [2 * BPC, S], f32, isOutput=True)

    with TileContext(nc) as tc:
        with (
            tc.tile_pool(name="const", bufs=1) as const,
            tc.tile_pool(name="inp", bufs=3) as inp,
            tc.tile_pool(name="hsb", bufs=4) as hsb,
            tc.tile_pool(name="pp", bufs=1) as pp,
            tc.tile_pool(name="wsum", bufs=2) as wsum,
            tc.tile_pool(name="ps_t", bufs=2, space="PSUM") as ps_t,
            tc.tile_pool(name="ps_h", bufs=3, space="PSUM") as ps_h,
            tc.tile_pool(name="ps_p", bufs=2, space="PSUM") as ps_p,
            tc.tile_pool(name="ps_w", bufs=1, space="PSUM") as ps_w,
        ):
            c_t = const.tile([128, 262], f32r)
            nc.sync.dma_start(out=c_t[:], in_=c_d[:])
            w1_t = c_t[:, 0:128]
            wc_t = c_t[:, 128 : 128 + 2 * BPC]
            id_t = c_t[:, 132:260]                    # [128, 128] f32r identity
            b1_t = c_t[:, 260:261].bitcast(f32)
            b2_t = c_t[:, 261:262].bitcast(f32)

            # Warm up the tensor engine's HAM clock gate while the first
            # loads are in flight, so real matmuls start at 2.4 GHz.
            wps = ps_w.tile([128, 256], f32)
            for _ in range(16):
                nc.tensor.matmul(
                    out=wps[:], lhsT=c_t[:, 0:128], rhs=c_t[:, 0:256],
                    start=True, stop=True,
                )

            # p_pad[(2b+f), 8 + s] = p^T; zero pads at both ends.
            p_pad = pp.tile([2 * BPC, PAD], f32)
            nc.vector.memset(p_pad[:, 0:8], 0.0)
            nc.vector.memset(p_pad[:, 8 + S : PAD], 0.0)

            for g in range(NG):
                s0 = g * GCHUNK
                xt = inp.tile([128, BPC, GCHUNK // 2], f32, tag="xt")
                pt = inp.tile([128, BPC, GCHUNK // 2], f32, tag="pt")
                HC = GCHUNK // 4
                for h in range(2):
                    for b in range(BPC):
                        nc.sync.dma_start(
                            out=xt[:, b, h * HC : (h + 1) * HC],
                            in_=x_d[b, s0 : s0 + GCHUNK, :].rearrange(
                                "(q v) f -> q (v f)", q=128
                            )[:, h * HC : (h + 1) * HC],
                        )
                        nc.sync.dma_start(
                            out=pt[:, b, h * HC : (h + 1) * HC],
                            in_=p_d[b, s0 : s0 + GCHUNK, :].rearrange(
                                "(q v) f -> q (v f)", q=128
                            )[:, h * HC : (h + 1) * HC],
                        )
                h0c = inp.tile([128, JG, BPC, I], f32r, tag="h0")
                for kk in range(GCHUNK // CHUNK):
                    js = slice(4 * kk, 4 * kk + 4)
                    nc.vector.tensor_add(
                        out=h0c[:, js, 0, :],
                        in0=xt[:, 0].rearrange("q (v f) -> q v f", f=I)[:, js],
                        in1=pt[:, 0].rearrange("q (v f) -> q v f", f=I)[:, js],
                    )
                    nc.gpsimd.tensor_add(
                        out=h0c[:, js, 1, :],
                        in0=xt[:, 1].rearrange("q (v f) -> q v f", f=I)[:, js],
                        in1=pt[:, 1].rearrange("q (v f) -> q v f", f=I)[:, js],
                    )

                for kk in range(GCHUNK // CHUNK):
                    h0T_ps = ps_t.tile([128, CHUNK], f32r)
                    for ul in range(CHUNK // 128):
                        u = 4 * kk + ul
                        nc.tensor.transpose(
                            out=h0T_ps[:, 128 * ul : 128 * ul + 128],
                            in_=h0c[:, u].rearrange("p b f -> p (b f)"),
                            identity=id_t[:],
                        )
                    h0T = hsb.tile([128, CHUNK], f32r, tag="h0T")
                    nc.vector.tensor_copy(out=h0T[:], in_=h0T_ps[:])

                    hT_ps = ps_h.tile([128, CHUNK], f32)
                    nc.tensor.matmul(
                        out=hT_ps[:], lhsT=w1_t, rhs=h0T[:], start=True, stop=True
                    )
                    hT = hsb.tile([128, CHUNK], f32r, tag="hT")
                    nc.scalar.activation(
                        out=hT[:],
                        in_=hT_ps[:],
                        func=mybir.ActivationFunctionType.Relu,
                        bias=b1_t,
                    )

                    pT_ps = ps_p.tile([2 * BPC, CHUNK], f32)
                    nc.tensor.matmul(
                        out=pT_ps[:], lhsT=wc_t, rhs=hT[:], start=True, stop=True
                    )
                    # Un-permute while scattering: pT col (ul, q) is seq
                    # s0 + 16q + 4kk + ul -> p_pad col base + 16q + ul.
                    base = 8 + s0 + 4 * kk
                    dst = p_pad[:, base : base + 2048].rearrange(
                        "p (pp u) -> p u pp", u=16
                    )[:, 0:4, :]
                    src = pT_ps.rearrange("p (u q) -> p u q", q=128)
                    if kk % 2 == 0:
                        nc.scalar.copy(out=dst, in_=src)
                    else:
                        nc.vector.tensor_copy(out=dst, in_=src)

            # Re-partition into halo layout Q[(bf,c), u] = p^T[bf, 128c+u-8]:
            # src iterates (bf, c, u), dest partition = 32*bf + c.
            q_t = wsum.tile([128, 144], f32, tag="q")
            nc.sync.dma_start(
                out=q_t[:, 0:128],
                in_=p_pad[:, 0 : 128 * 32].rearrange("p (c u) -> p c u", u=128),
            )
            nc.sync.dma_start(
                out=q_t[:, 128:136],
                in_=p_pad[:, 128 : 128 + 128 * 32].rearrange(
                    "p (c u) -> p c u", u=128
                )[:, :, 0:8],
            )
            nc.sync.dma_start(
                out=q_t[:, 136:144],
                in_=p_pad[:, 136 : 136 + 128 * 32].rearrange(
                    "p (c u) -> p c u", u=128
                )[:, :, 0:8],
            )

            # 17-wide window sum via doubling: ws[u] = sum_{d=0..16} Q[u+d].
            t2 = wsum.tile([128, 143], f32, tag="t2")
            nc.vector.tensor_add(out=t2[:], in0=q_t[:, 0:143], in1=q_t[:, 1:144])
            t4 = wsum.tile([128, 141], f32, tag="t4")
            nc.vector.tensor_add(out=t4[:], in0=t2[:, 0:141], in1=t2[:, 2:143])
            t8 = wsum.tile([128, 137], f32, tag="t8")
            nc.vector.tensor_add(out=t8[:], in0=t4[:, 0:137], in1=t4[:, 4:141])
            t16 = wsum.tile([128, 129], f32, tag="t16")
            nc.vector.tensor_add(out=t16[:], in0=t8[:, 0:129], in1=t8[:, 8:137])
            ws_t = wsum.tile([128, 128], f32, tag="ws")
            nc.vector.tensor_add(out=ws_t[:], in0=t16[:, 0:128], in1=q_t[:, 16:144])

            ows = wsum.tile([128, 128], f32, tag="ows")
            nc.scalar.activation(
                out=ows[:],
                in_=ws_t[:],
                func=mybir.ActivationFunctionType.Identity,
                bias=b2_t,
            )
            nc.sync.dma_start(
                out=o_d[:, :].rearrange("p (c u) -> (p c) u", u=128), in_=ows[:]
            )

    nc.finalize()
    return nc


def _get_program():
    global _PROGRAM
    if _PROGRAM is None:
        _PROGRAM = _build_program()
    return _PROGRAM


def _host_inputs(W1, b1, Wv, W2, b2):
    """Pack the small replicated parameters into one [128, 262] f32 tensor."""
    W1 = np.asarray(W1, np.float32).reshape(I, H)
    Wc = (
        np.asarray(Wv, np.float32).reshape(H, H)
        @ np.asarray(W2, np.float32).reshape(H, O)
    ).astype(np.float32)
    consts = np.zeros((128, 262), np.float32)
    consts[:64, 0:64] = W1
    consts[64:, 64:128] = W1
    consts[:64, 128 : 128 + O] = Wc
    consts[64:, 128 + O : 128 + 2 * O] = Wc
    consts[:, 132:260] = np.eye(128, dtype=np.float32)
    consts[:, 260] = np.tile(np.asarray(b1, np.float32).reshape(H), BPC)
    # winsum partition layout is (b, f, c): f = (p % 64) // 32
    pidx = np.arange(128)
    consts[:, 261] = np.asarray(b2, np.float32).reshape(O)[(pidx % 64) // 32]
    return consts


def _in_maps(x, pos_enc, W1, b1, Wv, W2, b2):
    x = np.asarray(x, np.float32)
    pos_enc = np.asarray(pos_enc, np.float32)
    consts = _host_inputs(W1, b1, Wv, W2, b2)
    in_maps = []
    for r in range(NCORES):
        sl = slice(r * BPC, (r + 1) * BPC)
        in_maps.append(
            {
                "x": np.ascontiguousarray(x[sl]),
                "p": np.ascontiguousarray(pos_enc[sl]),
                "consts": consts,
            }
        )
    return in_maps


def _assemble(results):
    out = np.empty((B, S, O), np.float32)
    for r in range(NCORES):
        o = np.asarray(results[r]["o"]).reshape(BPC, O, S)
        out[r * BPC : (r + 1) * BPC] = o.transpose(0, 2, 1)
    weights = np.ones((B, S, 1, 2 * A + 1), np.float32)
    return out, weights


def kernel(x, pos_enc, W1, b1, Wq, Wk, Wv, W2, b2):
    from concourse.bass_utils import run_bass_kernel_spmd

    in_maps = _in_maps(x, pos_enc, W1, b1, Wv, W2, b2)
    nc = _get_program()
    res = run_bass_kernel_spmd(nc, in_maps, list(range(NCORES))).results
    return _assemble(res)


# revision 41
# speedup vs baseline: 1.1731x; 1.0335x over previous
"""Trainium2 Bass kernel for a sliding-window self-attention block.

The reference network applies softmax over a singleton axis, so the attention
weights are identically 1.0 and the whole module reduces to:

    h   = relu((x + pos_enc) @ W1 + b1)            # [B, S, 64]
    p   = h @ (Wv @ W2)                            # [B, S, 2]
    out = sliding_window_sum(p, +-8, zero-pad) + b2
    weights = ones([B, S, 1, 17])

Sharding: data-parallel over batch, 2 batches per core on 8 NeuronCores.

Per-core dataflow (batches b0/b1 processed jointly on 128 partitions):
  - Inputs arrive feature-major ([B, I, S]; the host folds the transpose
    into the shard copy it makes anyway), so partition (b, f) tiles load
    with 16 KB-contiguous rows and no on-chip transpose is needed.
  - DVE/GpSimd adds -> h0^T in f32r, directly the matmul moving operand.
  - PE warmup matmuls run during the load phase to release the HAM clock
    gate before real work arrives.
  - One block-diagonal f32r matmul per 512-col chunk applies W1 to both
    batches; ACT relu(+b1); second block-diag matmul applies Wc = Wv@W2.
  - p^T chunks scatter (un-permuting seq) into a zero-padded [4, 4240]
    buffer; 3 SBUF DMAs re-partition it into a [(b,f,c), 144] halo
    layout; 5 log-tree DVE adds compute the 17-wide window sum; ACT adds
    b2; one DMA stores [4, 4096] = out^T per batch (host transposes).
"""

import numpy as np

B, S, I, H, O = 16, 4096, 64, 64, 2
A = 8                 # atten_size; window = 2*A+1 = 17
NCORES = 8
BPC = B // NCORES     # batches per core = 2
CHUNK = 512           # machine columns per PSUM-stage chunk
GCHUNK = 2048         # seq positions per load group
NG = S // GCHUNK      # 2
JG = GCHUNK // 128    # 16 u-slices per load group
PAD = 4240            # 8 zero pad + 4096 + 136 tail pad

_PROGRAM = None


def _build_program():
    import concourse.bacc as bacc
    import concourse.mybir as mybir
    from concourse.tile import TileContext

    f32 = mybir.dt.float32
    f32r = mybir.dt.float32r

    nc = bacc.Bacc()

    x_d = nc.declare_dram_parameter("x", [BPC, I, S], f32, isOutput=False)
    p_d = nc.declare_dram_parameter("p", [BPC, I, S], f32, isOutput=False)
    c_d = nc.declare_dram_parameter("consts", [128, 262], f32r, isOutput=False)
    o_d = nc.declare_dram_parameter("o", [2 * BPC, S], f32, isOutput=True)

    with TileContext(nc) as tc:
        with (
            tc.tile_pool(name="const", bufs=1) as const,
            tc.tile_pool(name="inp", bufs=3) as inp,
            tc.tile_pool(name="hsb", bufs=4) as hsb,
            tc.tile_pool(name="pp", bufs=1) as pp,
            tc.tile_pool(name="wsum", bufs=2) as wsum,
            tc.tile_pool(name="ps_t", bufs=2, space="PSUM") as ps_t,
            tc.tile_pool(name="ps_h", bufs=3, space="PSUM") as ps_h,
            tc.tile_pool(name="ps_p", bufs=2, space="PSUM") as ps_p,
            tc.tile_pool(name="ps_w", bufs=1, space="PSUM") as ps_w,
        ):
            c_t = const.tile([128, 262], f32r)
            nc.sync.dma_start(out=c_t[:], in_=c_d[:])
            w1_t = c_t[:, 0:128]
            wc_t = c_t[:, 128 : 128 + 2 * BPC]
            id_t = c_t[:, 132:260]                    # [128, 128] f32r identity
            b1_t = c_t[:, 260:261].bitcast(f32)
            b2_t = c_t[:, 261:262].bitcast(f32)

            # Warm up the tensor engine's HAM clock gate while the first
            # loads are in flight, so real matmuls start at 2.4 GHz.
            wps = ps_w.tile([128, 256], f32)
            for _ in range(16):
                nc.tensor.matmul(
                    out=wps[:], lhsT=c_t[:, 0:128], rhs=c_t[:, 0:256],
                    start=True, stop=True,
                )

            # p_pad[(2b+f), 8 + s] = p^T; zero pads at both ends.
            p_pad = pp.tile([2 * BPC, PAD], f32)
            nc.vector.memset(p_pad[:, 0:8], 0.0)
            nc.vector.memset(p_pad[:, 8 + S : PAD], 0.0)

            for g in range(NG):
                s0 = g * GCHUNK
                # Feature-major tiles: partition (b, f) = 64b + f, free = seq.
                # DRAM rows are 16 KB contiguous per partition -> minimal DMA
                # descriptor count; (b f) merges since b-stride = 64*f-stride.
                xt = inp.tile([128, GCHUNK], f32, tag="xt")
                pt = inp.tile([128, GCHUNK], f32, tag="pt")
                HC = GCHUNK // 2
                for h in range(2):
                    nc.sync.dma_start(
                        out=xt[:, h * HC : (h + 1) * HC],
                        in_=x_d[:, :, s0 + h * HC : s0 + (h + 1) * HC].rearrange(
                            "b f s -> (b f) s"
                        ),
                    )
                    nc.sync.dma_start(
                        out=pt[:, h * HC : (h + 1) * HC],
                        in_=p_d[:, :, s0 + h * HC : s0 + (h + 1) * HC].rearrange(
                            "b f s -> (b f) s"
                        ),
                    )
                # h0T = x + pos, already in the matmul rhs layout; adds split
                # per 512-col chunk between DVE and GpSimd.
                h0T = inp.tile([128, GCHUNK], f32r, tag="h0")
                for kk in range(GCHUNK // CHUNK):
                    cs = slice(CHUNK * kk, CHUNK * (kk + 1))
                    eng = nc.vector if kk % 2 == 0 else nc.gpsimd
                    eng.tensor_add(out=h0T[:, cs], in0=xt[:, cs], in1=pt[:, cs])

                for kk in range(GCHUNK // CHUNK):
                    cs = slice(CHUNK * kk, CHUNK * (kk + 1))
                    hT_ps = ps_h.tile([128, CHUNK], f32)
                    nc.tensor.matmul(
                        out=hT_ps[:], lhsT=w1_t, rhs=h0T[:, cs], start=True,
                        stop=True,
                    )
                    hT = hsb.tile([128, CHUNK], f32r, tag="hT")
                    nc.scalar.activation(
                        out=hT[:],
                        in_=hT_ps[:],
                        func=mybir.ActivationFunctionType.Relu,
                        bias=b1_t,
                    )

                    pT_ps = ps_p.tile([2 * BPC, CHUNK], f32)
                    nc.tensor.matmul(
                        out=pT_ps[:], lhsT=wc_t, rhs=hT[:], start=True, stop=True
                    )
                    # Columns are in natural seq order: contiguous scatter.
                    base = 8 + s0 + CHUNK * kk
                    if kk % 2 == 0:
                        nc.scalar.copy(
                            out=p_pad[:, base : base + CHUNK], in_=pT_ps[:]
                        )
                    else:
                        nc.vector.tensor_copy(
                            out=p_pad[:, base : base + CHUNK], in_=pT_ps[:]
                        )

            # Re-partition into halo layout Q[(bf,c), u] = p^T[bf, 128c+u-8]:
            # src iterates (bf, c, u), dest partition = 32*bf + c.
            q_t = wsum.tile([128, 144], f32, tag="q")
            nc.sync.dma_start(
                out=q_t[:, 0:128],
                in_=p_pad[:, 0 : 128 * 32].rearrange("p (c u) -> p c u", u=128),
            )
            nc.sync.dma_start(
                out=q_t[:, 128:136],
                in_=p_pad[:, 128 : 128 + 128 * 32].rearrange(
                    "p (c u) -> p c u", u=128
                )[:, :, 0:8],
            )
            nc.sync.dma_start(
                out=q_t[:, 136:144],
                in_=p_pad[:, 136 : 136 + 128 * 32].rearrange(
                    "p (c u) -> p c u", u=128
                )[:, :, 0:8],
            )

            # 17-wide window sum via doubling: ws[u] = sum_{d=0..16} Q[u+d].
            t2 = wsum.tile([128, 143], f32, tag="t2")
            nc.vector.tensor_add(out=t2[:], in0=q_t[:, 0:143], in1=q_t[:, 1:144])
            t4 = wsum.tile([128, 141], f32, tag="t4")
            nc.vector.tensor_add(out=t4[:], in0=t2[:, 0:141], in1=t2[:, 2:143])
            t8 = wsum.tile([128, 137], f32, tag="t8")
            nc.vector.tensor_add(out=t8[:], in0=t4[:, 0:137], in1=t4[:, 4:141])
            t16 = wsum.tile([128, 129], f32, tag="t16")
            nc.vector.tensor_add(out=t16[:], in0=t8[:, 0:129], in1=t8[:, 8:137])
            ws_t = wsum.tile([128, 128], f32, tag="ws")
            nc.vector.tensor_add(out=ws_t[:], in0=t16[:, 0:128], in1=q_t[:, 16:144])

            ows = wsum.tile([128, 128], f32, tag="ows")
            nc.scalar.activation(
                out=ows[:],
                in_=ws_t[:],
                func=mybir.ActivationFunctionType.Identity,
                bias=b2_t,
            )
            nc.sync.dma_start(
                out=o_d[:, :].rearrange("p (c u) -> (p c) u", u=128), in_=ows[:]
            )

    nc.finalize()
    return nc


def _get_program():
    global _PROGRAM
    if _PROGRAM is None:
        _PROGRAM = _build_program()
    return _PROGRAM


def _host_inputs(W1, b1, Wv, W2, b2):
    """Pack the small replicated parameters into one [128, 262] f32 tensor."""
    W1 = np.asarray(W1, np.float32).reshape(I, H)
    Wc = (
        np.asarray(Wv, np.float32).reshape(H, H)
        @ np.asarray(W2, np.float32).reshape(H, O)
    ).astype(np.float32)
    consts = np.zeros((128, 262), np.float32)
    consts[:64, 0:64] = W1
    consts[64:, 64:128] = W1
    consts[:64, 128 : 128 + O] = Wc
    consts[64:, 128 + O : 128 + 2 * O] = Wc
    consts[:, 132:260] = np.eye(128, dtype=np.float32)
    consts[:, 260] = np.tile(np.asarray(b1, np.float32).reshape(H), BPC)
    # winsum partition layout is (b, f, c): f = (p % 64) // 32
    pidx = np.arange(128)
    consts[:, 261] = np.asarray(b2, np.float32).reshape(O)[(pidx % 64) // 32]
    return consts


def _in_maps(x, pos_enc, W1, b1, Wv, W2, b2):
    x = np.asarray(x, np.float32)
    pos_enc = np.asarray(pos_enc, np.float32)
    consts = _host_inputs(W1, b1, Wv, W2, b2)
    in_maps = []
    for r in range(NCORES):
        sl = slice(r * BPC, (r + 1) * BPC)
        in_maps.append(
            {
                "x": np.ascontiguousarray(x[sl].transpose(0, 2, 1)),
                "p": np.ascontiguousarray(pos_enc[sl].transpose(0, 2, 1)),
                "consts": consts,
            }
        )
    return in_maps


def _assemble(results):
    out = np.empty((B, S, O), np.float32)
    for r in range(NCORES):
        o = np.asarray(results[r]["o"]).reshape(BPC, O, S)
        out[r * BPC : (r + 1) * BPC] = o.transpose(0, 2, 1)
    weights = np.ones((B, S, 1, 2 * A + 1), np.float32)
    return out, weights


def kernel(x, pos_enc, W1, b1, Wq, Wk, Wv, W2, b2):
    from concourse.bass_utils import run_bass_kernel_spmd

    in_maps = _in_maps(x, pos_enc, W1, b1, Wv, W2, b2)
    nc = _get_program()
    res = run_bass_kernel_spmd(nc, in_maps, list(range(NCORES))).results
    return _assemble(res)


# revision 45
# speedup vs baseline: 1.3046x; 1.1121x over previous
"""Trainium2 Bass kernel for a sliding-window self-attention block.

The reference network applies softmax over a singleton axis, so the attention
weights are identically 1.0 and the whole module reduces to:

    h   = relu((x + pos_enc) @ W1 + b1)            # [B, S, 64]
    p   = h @ (Wv @ W2)                            # [B, S, 2]
    out = sliding_window_sum(p, +-8, zero-pad) + b2
    weights = ones([B, S, 1, 17])

Sharding: data-parallel over batch, 2 batches per core on 8 NeuronCores.

Per-core dataflow (batches b0/b1 processed jointly on 128 partitions):
  - Inputs arrive feature-major ([B, I, S]; the host folds the transpose
    into the shard copy it makes anyway), so partition (b, f) tiles load
    with 16 KB-contiguous rows and no on-chip transpose is needed.
  - DVE/GpSimd adds -> h0^T in f32r, directly the matmul moving operand.
  - PE warmup matmuls run during the load phase to release the HAM clock
    gate before real work arrives.
  - One block-diagonal f32r matmul per 512-col chunk applies W1 to both
    batches; ACT relu(+b1); second block-diag matmul applies Wc = Wv@W2.
  - p^T chunks scatter (un-permuting seq) into a zero-padded [4, 4240]
    buffer; 3 SBUF DMAs re-partition it into a [(b,f,c), 144] halo
    layout; 5 log-tree DVE adds compute the 17-wide window sum; ACT adds
    b2; one DMA stores [4, 4096] = out^T per batch (host transposes).
"""

import numpy as np

B, S, I, H, O = 16, 4096, 64, 64, 2
A = 8                 # atten_size; window = 2*A+1 = 17
NCORES = 8
BPC = B // NCORES     # batches per core = 2
CHUNK = 512           # machine columns per PSUM-stage chunk
GCHUNK = 2048         # seq positions per load group
NG = S // GCHUNK      # 2
JG = GCHUNK // 128    # 16 u-slices per load group
PAD = 4240            # 8 zero pad + 4096 + 136 tail pad

_PROGRAM = None


def _build_program():
    import concourse.bacc as bacc
    import concourse.mybir as mybir
    from concourse.tile import TileContext

    f32 = mybir.dt.float32
    f32r = mybir.dt.float32r

    nc = bacc.Bacc()

    x_d = nc.declare_dram_parameter("x", [BPC, I, S], f32, isOutput=False)
    p_d = nc.declare_dram_parameter("p", [BPC, I, S], f32, isOutput=False)
    c_d = nc.declare_dram_parameter("consts", [128, 262], f32r, isOutput=False)
    o_d = nc.declare_dram_parameter("o", [2 * BPC, S], f32, isOutput=True)

    with TileContext(nc) as tc:
        with (
            tc.tile_pool(name="const", bufs=1) as const,
            tc.tile_pool(name="inp", bufs=3) as inp,
            tc.tile_pool(name="hsb", bufs=4) as hsb,
            tc.tile_pool(name="pp", bufs=1) as pp,
            tc.tile_pool(name="wsum", bufs=2) as wsum,
            tc.tile_pool(name="ps_t", bufs=2, space="PSUM") as ps_t,
            tc.tile_pool(name="ps_h", bufs=3, space="PSUM") as ps_h,
            tc.tile_pool(name="ps_p", bufs=2, space="PSUM") as ps_p,
            tc.tile_pool(name="ps_w", bufs=1, space="PSUM") as ps_w,
        ):
            c_t = const.tile([128, 262], f32r)
            nc.sync.dma_start(out=c_t[:], in_=c_d[:])
            w1_t = c_t[:, 0:128]
            wc_t = c_t[:, 128 : 128 + 2 * BPC]
            id_t = c_t[:, 132:260]                    # [128, 128] f32r identity
            b1_t = c_t[:, 260:261].bitcast(f32)
            b2_t = c_t[:, 261:262].bitcast(f32)

            # Warm up the tensor engine's HAM clock gate while the first
            # loads are in flight, so real matmuls start at 2.4 GHz.
            wps = ps_w.tile([128, 256], f32)
            for _ in range(16):
                nc.tensor.matmul(
                    out=wps[:], lhsT=c_t[:, 0:128], rhs=c_t[:, 0:256],
                    start=True, stop=True,
                )

            # p_pad[(2b+f), 8 + s] = p^T; zero pads at both ends.
            p_pad = pp.tile([2 * BPC, PAD], f32)
            nc.vector.memset(p_pad[:, 0:8], 0.0)
            nc.vector.memset(p_pad[:, 8 + S : PAD], 0.0)

            for g in range(NG):
                s0 = g * GCHUNK
                # Feature-major tiles: partition (b, f) = 64b + f, free = seq.
                # DRAM rows are 16 KB contiguous per partition -> minimal DMA
                # descriptor count; (b f) merges since b-stride = 64*f-stride.
                xt = inp.tile([128, GCHUNK], f32, tag="xt")
                pt = inp.tile([128, GCHUNK], f32, tag="pt")
                bounds = [0, 1024, 2048]
                for lo, hi in zip(bounds[:-1], bounds[1:]):
                    nc.sync.dma_start(
                        out=xt[:, lo:hi],
                        in_=x_d[:, :, s0 + lo : s0 + hi].rearrange(
                            "b f s -> (b f) s"
                        ),
                    )
                    nc.sync.dma_start(
                        out=pt[:, lo:hi],
                        in_=p_d[:, :, s0 + lo : s0 + hi].rearrange(
                            "b f s -> (b f) s"
                        ),
                    )
                # h0T = x + pos, already in the matmul rhs layout; adds split
                # per 512-col chunk between DVE and GpSimd.
                h0T = inp.tile([128, GCHUNK], f32r, tag="h0")
                for kk in range(GCHUNK // CHUNK):
                    cs = slice(CHUNK * kk, CHUNK * (kk + 1))
                    nc.vector.tensor_add(
                        out=h0T[:, cs], in0=xt[:, cs], in1=pt[:, cs]
                    )

                for kk in range(GCHUNK // CHUNK):
                    cs = slice(CHUNK * kk, CHUNK * (kk + 1))
                    hT_ps = ps_h.tile([128, CHUNK], f32)
                    nc.tensor.matmul(
                        out=hT_ps[:], lhsT=w1_t, rhs=h0T[:, cs], start=True,
                        stop=True,
                    )
                    hT = hsb.tile([128, CHUNK], f32r, tag="hT")
                    nc.scalar.activation(
                        out=hT[:],
                        in_=hT_ps[:],
                        func=mybir.ActivationFunctionType.Relu,
                        bias=b1_t,
                    )

                    pT_ps = ps_p.tile([2 * BPC, CHUNK], f32)
                    nc.tensor.matmul(
                        out=pT_ps[:], lhsT=wc_t, rhs=hT[:], start=True, stop=True
                    )
                    # Columns are in natural seq order: contiguous scatter.
                    base = 8 + s0 + CHUNK * kk
                    nc.scalar.copy(
                        out=p_pad[:, base : base + CHUNK], in_=pT_ps[:]
                    )

            # Re-partition into halo layout Q[(bf,c), u] = p^T[bf, 128c+u-8]:
            # src iterates (bf, c, u), dest partition = 32*bf + c.
            q_t = wsum.tile([128, 144], f32, tag="q")
            nc.scalar.dma_start(
                out=q_t[:, 0:128],
                in_=p_pad[:, 0 : 128 * 32].rearrange("p (c u) -> p c u", u=128),
            )
            nc.sync.dma_start(
                out=q_t[:, 128:136],
                in_=p_pad[:, 128 : 128 + 128 * 32].rearrange(
                    "p (c u) -> p c u", u=128
                )[:, :, 0:8],
            )
            nc.sync.dma_start(
                out=q_t[:, 136:144],
                in_=p_pad[:, 136 : 136 + 128 * 32].rearrange(
                    "p (c u) -> p c u", u=128
                )[:, :, 0:8],
            )

            # 17-wide window sum via doubling: ws[u] = sum_{d=0..16} Q[u+d].
            t2 = wsum.tile([128, 143], f32, tag="t2")
            nc.vector.tensor_add(out=t2[:], in0=q_t[:, 0:143], in1=q_t[:, 1:144])
            t4 = wsum.tile([128, 141], f32, tag="t4")
            nc.vector.tensor_add(out=t4[:], in0=t2[:, 0:141], in1=t2[:, 2:143])
            t8 = wsum.tile([128, 137], f32, tag="t8")
            nc.vector.tensor_add(out=t8[:], in0=t4[:, 0:137], in1=t4[:, 4:141])
            t16 = wsum.tile([128, 129], f32, tag="t16")
            nc.vector.tensor_add(out=t16[:], in0=t8[:, 0:129], in1=t8[:, 8:137])
            ws_t = wsum.tile([128, 128], f32, tag="ws")
            nc.vector.tensor_add(out=ws_t[:], in0=t16[:, 0:128], in1=q_t[:, 16:144])

            ows = wsum.tile([128, 128], f32, tag="ows")
            nc.scalar.activation(
                out=ows[:],
                in_=ws_t[:],
                func=mybir.ActivationFunctionType.Identity,
                bias=b2_t,
            )
            nc.sync.dma_start(
                out=o_d[:, :].rearrange("p (c u) -> (p c) u", u=128), in_=ows[:]
            )

    nc.finalize()
    return nc


def _get_program():
    global _PROGRAM
    if _PROGRAM is None:
        _PROGRAM = _build_program()
    return _PROGRAM


def _host_inputs(W1, b1, Wv, W2, b2):
    """Pack the small replicated parameters into one [128, 262] f32 tensor."""
    W1 = np.asarray(W1, np.float32).reshape(I, H)
    Wc = (
        np.asarray(Wv, np.float32).reshape(H, H)
        @ np.asarray(W2, np.float32).reshape(H, O)
    ).astype(np.float32)
    consts = np.zeros((128, 262), np.float32)
    consts[:64, 0:64] = W1
    consts[64:, 64:128] = W1
    consts[:64, 128 : 128 + O] = Wc
    consts[64:, 128 + O : 128 + 2 * O] = Wc
    consts[:, 132:260] = np.eye(128, dtype=np.float32)
    consts[:, 260] = np.tile(np.asarray(b1, np.float32).reshape(H), BPC)
    # winsum partition layout is (b, f, c): f = (p % 64) // 32
    pidx = np.arange(128)
    consts[:, 261] = np.asarray(b2, np.float32).reshape(O)[(pidx % 64) // 32]
    return consts


def _in_maps(x, pos_enc, W1, b1, Wv, W2, b2):
    x = np.asarray(x, np.float32)
    pos_enc = np.asarray(pos_enc, np.float32)
    consts = _host_inputs(W1, b1, Wv, W2, b2)
    in_maps = []
    for r in range(NCORES):
        sl = slice(r * BPC, (r + 1) * BPC)
        in_maps.append(
            {
                "x": np.ascontiguousarray(x[sl].transpose(0, 2, 1)),
                "p": np.ascontiguousarray(pos_enc[sl].transpose(0, 2, 1)),
                "consts": consts,
            }
        )
    return in_maps


def _assemble(results):
    out = np.empty((B, S, O), np.float32)
    for r in range(NCORES):
        o = np.asarray(results[r]["o"]).reshape(BPC, O, S)
        out[r * BPC : (r + 1) * BPC] = o.transpose(0, 2, 1)
    weights = np.ones((B, S, 1, 2 * A + 1), np.float32)
    return out, weights


def kernel(x, pos_enc, W1, b1, Wq, Wk, Wv, W2, b2):
    from concourse.bass_utils import run_bass_kernel_spmd

    in_maps = _in_maps(x, pos_enc, W1, b1, Wv, W2, b2)
    nc = _get_program()
    res = run_bass_kernel_spmd(nc, in_maps, list(range(NCORES))).results
    return _assemble(res)
